# revision 1
# baseline (speedup 1.0000x reference)
"""Trainium2 Bass kernel for nn_Attention_Module (sparse_attention).

Computation per batch b (x_b: [C=256, T=4096] fp32):
    energy = x_b @ x_b^T                      # (256, 256), K=4096
    attn   = softmax(rowmax(energy) - energy) # == exp(mu - e)/Z, mu = rowmin
    out    = gamma * (attn @ x_b) + x_b

Strategy (8 cores, pure data-parallel, 4 batches/core):
  - Host pre-transposes x -> xT so the energy matmul (contraction over T)
    gets t-partition tiles with contiguous DMA; x natural layout feeds the
    second matmul (contraction over C) and the +x residual.
  - Matmuls run as float32r (fp22 multiplies, fp32 accumulate).
  - The +x residual is folded into matmul2 via the modified attention matrix
    A'' = gamma*P^T + diag(Z); out = diag(1/Z) * (A''^T @ x). Using the same
    Z in diag and the post-scale makes the x term exact.
  - Software-pipelined DMA issue: batch b+1 loads are issued before batch b
    stores so the HWDGE ring never blocks loads behind compute-gated stores.
    Whole-tensor transfers (4 MB) keep DMA at peak efficiency.
"""

import numpy as np

B, C, T = 32, 256, 4096
NCORES = 8
NB = B // NCORES  # batches per core
P = 128
KT = T // P  # 32 k-tiles for the energy matmul
TC = T // 512  # 8 t-chunks for matmul2

_CACHE = {}


def _build_nc(variant=None):
    variant = variant or {}
    from contextlib import ExitStack

    import concourse.bacc as bacc
    import concourse.bass as bass
    import concourse.tile as tile
    from concourse import mybir

    f32 = mybir.dt.float32
    f32r = mybir.dt.float32r
    ts = bass.ts

    nc = bacc.Bacc(
        "TRN2",
        target_bir_lowering=False,
        debug=False,
        enable_asserts=False,
        num_devices=NCORES,
    )

    xt_h = nc.dram_tensor("xt", [NB, T, C], f32r, kind="ExternalInput")
    xn_h = nc.dram_tensor("xn", [NB, C, T], mybir.dt.float16, kind="ExternalInput")
    # aux: per-partition row [gamma, pad, pad, pad, identity-row(128)]
    aux_h = nc.dram_tensor("aux", [P, 132], f32, kind="ExternalInput")
    o_h = nc.dram_tensor("o", [NB, C, T], f32, kind="ExternalOutput")

    with tile.TileContext(nc) as tc:
        with ExitStack() as ctx:
            singles = ctx.enter_context(tc.tile_pool(name="singles", bufs=1))
            xt_pool = ctx.enter_context(
                tc.tile_pool(
                    name="xt", bufs=3 if variant.get("xt_bufs3") else 2
                )
            )
            xq_pool = (
                ctx.enter_context(tc.tile_pool(name="xq", bufs=1))
                if variant.get("fast_start")
                else None
            )
            xn_pool = ctx.enter_context(tc.tile_pool(name="xn", bufs=3))
            out_pool = ctx.enter_context(tc.tile_pool(name="out", bufs=2))
            att_pool = ctx.enter_context(tc.tile_pool(name="att", bufs=3))
            small = ctx.enter_context(tc.tile_pool(name="small", bufs=4))
            psum_e = ctx.enter_context(
                tc.tile_pool(name="psum_e", bufs=2, space="PSUM")
            )
            psum_t = ctx.enter_context(
                tc.tile_pool(name="psum_t", bufs=2, space="PSUM")
            )
            psum_o = ctx.enter_context(
                tc.tile_pool(
                    name="psum_o",
                    bufs=6 if variant.get("xbar_pt") else 4,
                    space="PSUM",
                )
            )

            xt_ap = xt_h.ap()
            xn_ap = xn_h.ap()
            o_ap = o_h.ap()

            # aux on the ACT ring so it doesn't delay the first xt load
            aux = singles.tile([P, 132], f32)
            nc.scalar.dma_start(aux[:], aux_h.ap())
            gv = aux[:, 0:1]
            ivg = aux[:, 1:2]
            ident = aux[:, 4:132]

            def issue_loads(b):
                KH = KT // 2
                xt_r = xt_ap[b].rearrange("(k p) c -> p k c", p=P)
                if variant.get("fast_start") and b == 0:
                    # four separate tiles so matmul1 starts after the first
                    # 1 MB lands (Tile tracks deps at tile granularity)
                    KQ = KT // 4
                    qs = []
                    for q in range(4):
                        t_ = xq_pool.tile(
                            [P, KQ, C], f32r, tag=f"xq{q}", name=f"xq{q}"
                        )
                        nc.sync.dma_start(
                            t_[:], xt_r[:, q * KQ : (q + 1) * KQ, :]
                        )
                        qs.append(t_)
                    xt_tiles, kdiv = qs, KQ
                else:
                    xta = xt_pool.tile([P, KH, C], f32r, tag="xta", name="xta")
                    xtb = xt_pool.tile([P, KH, C], f32r, tag="xtb", name="xtb")
                    if variant.get("fine_ends") and b == 0:
                        for t_, off in ((xta, 0), (xtb, KH)):
                            for q in range(2):
                                nc.sync.dma_start(
                                    t_[:, q * (KH // 2) : (q + 1) * (KH // 2), :],
                                    xt_r[
                                        :,
                                        off
                                        + q * (KH // 2) : off
                                        + (q + 1) * (KH // 2),
                                        :,
                                    ],
                                )
                    else:
                        nc.sync.dma_start(xta[:], xt_r[:, :KH, :])
                        nc.sync.dma_start(xtb[:], xt_r[:, KH:, :])
                    xt_tiles, kdiv = [xta, xtb], KH
                xn = xn_pool.tile([P, 2, T], mybir.dt.float16, tag="xn", name="xn")
                nc.sync.dma_start(
                    xn[:], xn_ap[b].rearrange("(m p) t -> p m t", p=P)
                )
                return xt_tiles, kdiv, xn

            tiles = {0: issue_loads(0)}
            pending = None  # (b, At, rZ, xn) awaiting matmul2

            for b in range(NB):
                xt, kdiv, xn = tiles.pop(b)
                if b + 1 < NB:
                    tiles[b + 1] = issue_loads(b + 1)

                # A''^T, laid out [128(j within k-block), k-block, 256(i)]
                At = att_pool.tile(
                    [P, 2, C],
                    mybir.dt.bfloat16
                    if variant.get("xbar_pt")
                    else mybir.dt.float16,
                )
                Zs = small.tile([P, 2], f32)
                Zb = small.tile(
                    [P, 2],
                    mybir.dt.bfloat16
                    if variant.get("xbar_pt")
                    else mybir.dt.float16,
                )
                rZ = small.tile([P, 2], f32)

                for m in range(2):
                    pe = psum_e.tile([P, C], mybir.dt.float32)
                    for k in range(KT):
                        src_t = xt[k // kdiv]
                        kk = k % kdiv
                        nc.tensor.matmul(
                            pe[:],
                            lhsT=src_t[:, kk, ts(m, P)],
                            rhs=src_t[:, kk, :],
                            start=(k == 0),
                            stop=(k == KT - 1),
                        )
                    mu = small.tile([P, 1], f32)
                    nc.vector.tensor_reduce(
                        mu[:], pe[:], axis=mybir.AxisListType.X,
                        op=mybir.AluOpType.min,
                    )
                    if variant.get("xbar_pt"):
                        # P in bf16; transpose via xbar DMA on the ACT ring.
                        # A''T = P^T + diag(W), W = Z/gamma; out scale = 1/W.
                        Pm = small.tile([P, C], mybir.dt.bfloat16, tag="Pm")
                        nc.scalar.activation(
                            Pm[:],
                            pe[:],
                            mybir.ActivationFunctionType.Exp,
                            bias=mu[:],
                            scale=-1.0,
                            accum_out=Zs[:, m : m + 1],
                        )
                        Wv = small.tile([P, 2], f32, tag="Wv", name="Wv")
                        nc.vector.tensor_scalar_mul(
                            Wv[:, m : m + 1], Zs[:, m : m + 1], ivg
                        )
                        nc.vector.tensor_copy(Zb[:, m : m + 1], Wv[:, m : m + 1])
                        nc.vector.reciprocal(rZ[:, m : m + 1], Zb[:, m : m + 1])
                        for k in range(2):
                            nc.scalar.dma_start_transpose(
                                At[:, k, ts(m, P)], Pm[:, ts(k, P)]
                            )
                        dg = small.tile([P, P], mybir.dt.bfloat16, tag="diag")
                        nc.vector.tensor_scalar_mul(dg[:], ident, Wv[:, m : m + 1])
                        nc.vector.tensor_add(
                            At[:, m, ts(m, P)], At[:, m, ts(m, P)], dg[:]
                        )
                    else:
                        Pm = small.tile([P, C], f32, tag="Pm")
                        nc.scalar.activation(
                            Pm[:],
                            pe[:],
                            mybir.ActivationFunctionType.Exp,
                            bias=mu[:],
                            scale=-1.0,
                            accum_out=Zs[:, m : m + 1],
                        )
                        nc.vector.tensor_copy(Zb[:, m : m + 1], Zs[:, m : m + 1])
                        nc.vector.reciprocal(rZ[:, m : m + 1], Zb[:, m : m + 1])
                        for k in range(2):
                            pt = psum_t.tile([P, P], mybir.dt.float32)
                            nc.tensor.transpose(pt[:], Pm[:, ts(k, P)], ident)
                            # A''T[j in k-block, i in m-block] = gamma * P^T
                            nc.scalar.mul(At[:, k, ts(m, P)], pt[:], gv)
                        # diagonal: += diag(Z) (falls in the k == m block)
                        dg = small.tile([P, P], mybir.dt.float16, tag="diag")
                        nc.vector.tensor_scalar_mul(dg[:], ident, Zs[:, m : m + 1])
                        nc.vector.tensor_add(
                            At[:, m, ts(m, P)], At[:, m, ts(m, P)], dg[:]
                        )

                # software-pipeline the PE: run the PREVIOUS batch's matmul2
                # after this batch's matmul1, hiding the A'' build latency.
                this = (b, At, rZ, xn)
                todo = [pending] if pending is not None else []
                if b == NB - 1:
                    todo.append(this)
                    pending = None
                else:
                    pending = this
                for pb, pAt, prZ, pxn in todo:
                    for m in range(2):
                        ot = out_pool.tile([P, T], f32, tag="ot", name="ot")
                        if variant.get("mm2_wruns"):
                            # weight-consecutive runs of 2 so walrus ldw-opt
                            # can elide redundant embedded weight loads
                            for g in range(TC // 2):
                                pos = [
                                    psum_o.tile(
                                        [P, 512], mybir.dt.float32,
                                        name=f"po{j}", tag="po",
                                    )
                                    for j in range(2)
                                ]
                                for k in range(2):
                                    for j in range(2):
                                        nc.tensor.matmul(
                                            pos[j][:],
                                            lhsT=pAt[:, k, ts(m, P)],
                                            rhs=pxn[:, k, ts(2 * g + j, 512)],
                                            start=(k == 0),
                                            stop=(k == 1),
                                        )
                                for j in range(2):
                                    t8 = 2 * g + j
                                    if t8 % 2 == 0:
                                        nc.vector.tensor_scalar_mul(
                                            ot[:, ts(t8, 512)], pos[j][:],
                                            prZ[:, m : m + 1],
                                        )
                                    else:
                                        nc.scalar.mul(
                                            ot[:, ts(t8, 512)], pos[j][:],
                                            prZ[:, m : m + 1],
                                        )
                        else:
                            for t8 in range(TC):
                                po = psum_o.tile([P, 512], mybir.dt.float32)
                                for k in range(2):
                                    nc.tensor.matmul(
                                        po[:],
                                        lhsT=pAt[:, k, ts(m, P)],
                                        rhs=pxn[:, k, ts(t8, 512)],
                                        start=(k == 0),
                                        stop=(k == 1),
                                    )
                                # out = psum * (1/Z); alternate engines
                                if t8 % 2 == 0:
                                    nc.vector.tensor_scalar_mul(
                                        ot[:, ts(t8, 512)], po[:], prZ[:, m : m + 1]
                                    )
                                else:
                                    nc.scalar.mul(
                                        ot[:, ts(t8, 512)], po[:], prZ[:, m : m + 1]
                                    )
                        nsplit = (
                            4
                            if (variant.get("fine_ends") or variant.get("fast_start"))
                            and pb == NB - 1
                            else 2
                        )
                        store_eng = (
                            nc.scalar if variant.get("act_stores") else nc.sync
                        )
                        for sh in range(nsplit):
                            store_eng.dma_start(
                                o_ap[pb].rearrange("(m p) t -> p m t", p=P)[
                                    :, m, ts(sh, T // nsplit)
                                ],
                                ot[:, ts(sh, T // nsplit)],
                            )

    nc.compile()
    return nc


def _get_nc():
    if "nc" not in _CACHE:
        _CACHE["nc"] = _build_nc()
    return _CACHE["nc"]


def _make_aux(gamma_val):
    aux = np.zeros((P, 132), dtype=np.float32)
    aux[:, 0] = gamma_val
    aux[:, 1] = 1.0 / gamma_val if gamma_val != 0 else 0.0
    aux[:, 4:132] = np.eye(P, dtype=np.float32)
    return aux


def kernel(x, gamma, _trace=False):
    import concourse.bass_utils as bass_utils

    x = np.ascontiguousarray(np.asarray(x, dtype=np.float32))
    gamma = np.asarray(gamma, dtype=np.float32).reshape(-1)

    nc = _get_nc()

    aux = _make_aux(gamma[0])
    in_maps = []
    for d in range(NCORES):
        xs = x[d * NB : (d + 1) * NB]
        in_maps.append(
            {
                "xt": np.ascontiguousarray(xs.transpose(0, 2, 1)),
                "xn": xs.astype(np.float16),
                "aux": aux,
            }
        )

    res = bass_utils.run_bass_kernel_spmd(
        nc, in_maps, core_ids=list(range(NCORES)), trace=_trace
    )
    out = np.concatenate([r["o"] for r in res.results], axis=0)
    if _trace:
        _CACHE["last_results"] = res
    return out



# revision 9
# speedup vs baseline: 1.2608x; 1.2608x over previous
"""Trainium2 Bass kernel for nn_Attention_Module (sparse_attention).

Computation per batch b (x_b: [C=256, T=4096] fp32):
    energy = x_b @ x_b^T                      # (256, 256), K=4096
    attn   = softmax(rowmax(energy) - energy) # == exp(mu - e)/Z, mu = rowmin
    out    = gamma * (attn @ x_b) + x_b

Strategy (8 cores, pure data-parallel, 4 batches/core):
  - x is loaded from HBM exactly ONCE, as fp16 in t-major layout
    (xt[b, p, k, c] = x[b, c, k*128+p]) with fat 8KB/partition DMA lines.
    The c-major copy needed by matmul2 (xn) is derived on-chip with PE
    transposes (fp16, 1 cycle/row) instead of a second HBM load.
    DMA drops from 41.9 MB/core (baseline) to 25.2 MB/core.
  - B = gamma*diag(1/Z)*P + I is materialized directly as the matmul2
    weights: P rows are scaled by 1/Z (per-partition) BEFORE the PE
    transpose, gamma is applied during the PSUM->SBUF copy of P^T, and
    the +x residual is the exact identity diagonal. matmul2 then yields
    the final output with no post-scale (plain PSUM->SBUF copies).
  - Copies are round-robined across ACT/DVE/Pool so no single engine
    bottlenecks; all DMA is issued from SP.
  - Software pipeline: batch b's slot runs mm2(b-1), mm1(b), xn
    transposes(b), At(b); loads for b+1 are issued up front.
"""

import numpy as np

B, C, T = 32, 256, 4096
NCORES = 8
NB = B // NCORES  # batches per core
P = 128
KT = T // P  # 32 t-tiles of 128
KH = KT // 2  # 16 per half-tile
TC = T // 512  # 8 output chunks per m-block

_CACHE = {}


def _build_nc(variant=None):
    variant = variant or {}
    from contextlib import ExitStack

    import concourse.bacc as bacc
    import concourse.bass as bass
    import concourse.tile as tile
    from concourse import mybir

    f32 = mybir.dt.float32
    f16 = mybir.dt.float16
    ts = bass.ts

    nc = bacc.Bacc(
        "TRN2",
        target_bir_lowering=False,
        debug=False,
        enable_asserts=False,
        num_devices=NCORES,
    )

    # xt[b, p, k, c] = x[b, c, k*128+p]; per-partition lines are 8KB.
    xt_h = nc.dram_tensor("xt", [NB, P, KT, C], f16, kind="ExternalInput")
    # aux: [gamma(f32)] broadcast over partitions
    aux_h = nc.dram_tensor("aux", [P, 4], f32, kind="ExternalInput")
    idn_h = nc.dram_tensor("idn", [P, P], f16, kind="ExternalInput")
    o_h = nc.dram_tensor("o", [NB, C, T], f32, kind="ExternalOutput")

    with tile.TileContext(nc) as tc:
        with ExitStack() as ctx:
            singles = ctx.enter_context(tc.tile_pool(name="singles", bufs=1))
            xt_pool = ctx.enter_context(tc.tile_pool(name="xt", bufs=2))
            xn_pool = ctx.enter_context(tc.tile_pool(name="xn", bufs=2))
            out_pool = ctx.enter_context(tc.tile_pool(name="out", bufs=2))
            att_pool = ctx.enter_context(tc.tile_pool(name="att", bufs=2))
            small = ctx.enter_context(tc.tile_pool(name="small", bufs=3))
            psum_e = ctx.enter_context(
                tc.tile_pool(name="psum_e", bufs=2, space="PSUM")
            )
            psum_x = ctx.enter_context(
                tc.tile_pool(name="psum_x", bufs=2, space="PSUM")
            )
            psum_o = ctx.enter_context(
                tc.tile_pool(name="psum_o", bufs=4, space="PSUM")
            )

            xt_ap = xt_h.ap()
            o_ap = o_h.ap()

            aux = singles.tile([P, 4], f32)
            nc.scalar.dma_start(aux[:], aux_h.ap())
            idn = singles.tile([P, P], f16)
            nc.scalar.dma_start(idn[:], idn_h.ap())
            gv = aux[:, 0:1]

            def issue_loads(b):
                xta = xt_pool.tile([P, KH, C], f16, tag="xta", name="xta")
                xtb = xt_pool.tile([P, KH, C], f16, tag="xtb", name="xtb")
                if b == 0:
                    # split first tile's DMA so mm1 can start sooner
                    for t_, off in ((xta, 0), (xtb, KH)):
                        for q in range(2):
                            nc.sync.dma_start(
                                t_[:, q * (KH // 2) : (q + 1) * (KH // 2), :],
                                xt_ap[
                                    b,
                                    :,
                                    off + q * (KH // 2) : off + (q + 1) * (KH // 2),
                                    :,
                                ],
                            )
                else:
                    nc.sync.dma_start(xta[:], xt_ap[b, :, :KH, :])
                    nc.sync.dma_start(xtb[:], xt_ap[b, :, KH:, :])
                return (xta, xtb)

            # round-robin copy engines for PSUM->SBUF drains
            cp_engines = [
                lambda o, i: nc.scalar.copy(o, i),
                lambda o, i: nc.vector.tensor_copy(o, i),
            ]
            cp_idx = [0]

            def copy_eng():
                e = cp_engines[cp_idx[0] % len(cp_engines)]
                cp_idx[0] += 1
                return e

            def run_mm2(pb, pAt, pxn):
                """out(pb) = B^T-weighted matmul2 + stores (final values)."""
                nsplit = 4 if pb == NB - 1 else 2
                for m in range(2):
                    ot = out_pool.tile([P, T], f32, tag="ot", name="ot")
                    for t8 in range(TC):
                        po = psum_o.tile([P, 512], f32)
                        for k in range(2):
                            nc.tensor.matmul(
                                po[:],
                                lhsT=pAt[:, k, ts(m, P)],
                                rhs=pxn[:, k, ts(t8, 512)],
                                start=(k == 0),
                                stop=(k == 1),
                            )
                        copy_eng()(ot[:, ts(t8, 512)], po[:])
                    for sh in range(nsplit):
                        nc.sync.dma_start(
                            o_ap[pb].rearrange("(m p) t -> p m t", p=P)[
                                :, m, ts(sh, T // nsplit)
                            ],
                            ot[:, ts(sh, T // nsplit)],
                        )

            tiles = {0: issue_loads(0)}
            pending = None  # (b, At, xn) awaiting matmul2

            for b in range(NB):
                xth = tiles.pop(b)
                if b + 1 < NB:
                    tiles[b + 1] = issue_loads(b + 1)

                # run the PREVIOUS batch's matmul2 first: its deps (At, xn)
                # resolved last slot, so the PE never stalls on softmax.
                if pending is not None:
                    run_mm2(*pending)
                    pending = None

                xn = xn_pool.tile([P, 2, T], f16, tag="xn", name="xn")
                At = att_pool.tile([P, 2, C], f16)
                Zs = small.tile([P, 2], f32, tag="Zs")
                rZ = small.tile([P, 2], f32, tag="rZ")

                for m in range(2):
                    # ---- mm1: energy rows for i in m-block ----
                    pe = psum_e.tile([P, C], f32)
                    for k in range(KT):
                        src = xth[k // KH]
                        kk = k % KH
                        nc.tensor.matmul(
                            pe[:],
                            lhsT=src[:, kk, ts(m, P)],
                            rhs=src[:, kk, :],
                            start=(k == 0),
                            stop=(k == KT - 1),
                        )

                    # ---- xn transposes for half m (keeps PE busy while
                    # softmax latency resolves on ACT/DVE) ----
                    src = xth[m]
                    for g in range(KH // 4):  # 4 groups of 4 k-tiles
                        for cb in range(2):
                            px = psum_x.tile([P, 512], f16)
                            for j in range(4):
                                nc.tensor.transpose(
                                    px[:, ts(j, P)],
                                    src[:, g * 4 + j, ts(cb, P)],
                                    idn[:],
                                )
                            tb = (m * KH + g * 4) * P
                            copy_eng()(xn[:, cb, tb : tb + 512], px[:])

                    # ---- softmax -> scaled weights ----
                    mu = small.tile([P, 1], f32, tag="mu")
                    nc.vector.tensor_reduce(
                        mu[:], pe[:], axis=mybir.AxisListType.X,
                        op=mybir.AluOpType.min,
                    )
                    Pm = small.tile([P, C], f32, tag="Pm")
                    nc.scalar.activation(
                        Pm[:],
                        pe[:],
                        mybir.ActivationFunctionType.Exp,
                        bias=mu[:],
                        scale=-1.0,
                        accum_out=Zs[:, m : m + 1],
                    )
                    nc.vector.reciprocal(rZ[:, m : m + 1], Zs[:, m : m + 1])
                    Pm2 = small.tile([P, C], f16, tag="Pm2")
                    nc.vector.tensor_scalar_mul(Pm2[:], Pm[:], rZ[:, m : m + 1])

                    # ---- At[:, k2, m-block] = gamma * (P/Z)^T (+ I on diag) ----
                    ptf = psum_x.tile([P, 512], f16, tag="px", name="ptf")
                    pt = ptf[:, :C]
                    for k2 in range(2):
                        nc.tensor.transpose(
                            pt[:, ts(k2, P)], Pm2[:, ts(k2, P)], idn[:]
                        )
                    nc.scalar.mul(At[:, :, ts(m, P)], pt[:], gv)
                    nc.vector.tensor_add(
                        At[:, m, ts(m, P)], At[:, m, ts(m, P)], idn[:]
                    )

                this = (b, At, xn)
                if b == NB - 1:
                    run_mm2(*this)
                else:
                    pending = this

    nc.compile()
    return nc


def _get_nc():
    if "nc" not in _CACHE:
        _CACHE["nc"] = _build_nc()
    return _CACHE["nc"]


def _make_aux(gamma_val):
    aux = np.zeros((P, 4), dtype=np.float32)
    aux[:, 0] = gamma_val
    return aux


def kernel(x, gamma, _trace=False):
    import concourse.bass_utils as bass_utils

    x = np.ascontiguousarray(np.asarray(x, dtype=np.float32))
    gamma = np.asarray(gamma, dtype=np.float32).reshape(-1)

    nc = _get_nc()

    aux = _make_aux(gamma[0])
    idn = np.eye(P, dtype=np.float16)
    # xt[b, p, k, c] = x[b, c, k*128+p]
    xt_all = (
        x.astype(np.float16)
        .reshape(B, C, KT, P)
        .transpose(0, 3, 2, 1)
    )
    in_maps = []
    for d in range(NCORES):
        in_maps.append(
            {
                "xt": np.ascontiguousarray(xt_all[d * NB : (d + 1) * NB]),
                "aux": aux,
                "idn": idn,
            }
        )

    res = bass_utils.run_bass_kernel_spmd(
        nc, in_maps, core_ids=list(range(NCORES)), trace=_trace
    )
    out = np.concatenate([r["o"] for r in res.results], axis=0)
    if _trace:
        _CACHE["last_results"] = res
    return out
